# revision 2
# baseline (speedup 1.0000x reference)
"""MultiProbeAttentionPooler Trainium2 kernel.

  logits[b,t,p] = x[b,t,:] @ W[p,:] + b[p]
  att = softmax(logits, axis=t)          -> returned as [B, P, T]
  pooled[b,p,d] = sum_t att[b,t,p] x[b,t,d]

Sharding: T is split across the 8 cores (512 tokens each); every core
handles all 16 batches.  Softmax normalization is deferred to the host:
each core returns unnormalized exp(logits) tiles, per-probe partial sums
Z, and unnormalized partial pooled sums; the host reduces/divides.

Per core, per (batch, T-shard) unit of work:
  - DMA x slice [512, 1024] fp32 naturally (T on partitions).
  - TensorE transposes build xT tiles [D on partitions] (float32r,
    1.5 cy/row) for the logits matmul; PSUM->SBUF copies alternate
    between ScalarE and VectorE.
  - mm1 (float32r): logitsT[p, t] accumulated over 8 D-chunks in PSUM.
  - ScalarE Exp with per-partition bias adds b and produces E plus its
    row sums (accum_out) in one pass.
  - E chunks are transposed back to [t, p] as the stationary operand of
  - mm2 (float32r): pooled_raw[p, d] accumulated over 4 T-blocks.

float32r runs the PE at 1 cycle/row (vs 4 for fp32) at ~1.3e-4 relative
precision; fp32 would be ~3x slower than the HBM roofline.
"""

import sys

import numpy as np

for _p in ("/opt/trn_rl_repo", "/root/.axon_site/_ro/trn_rl_repo"):
    if _p not in sys.path:
        sys.path.append(_p)

import concourse.bacc as bacc
import concourse.tile as tile
from concourse import mybir
from concourse.bass_utils import run_bass_kernel_spmd

dt = mybir.dt
AF = mybir.ActivationFunctionType

B, T, D, P = 16, 4096, 1024, 16
N_CORES = 8
TSH = T // N_CORES          # tokens per core (512)
TBLK = TSH // 128           # 128-token blocks per unit (4)
DSUB = D // 128             # 128-wide D chunks (8)

_CACHE = {}


def _build():
    nc = bacc.Bacc("TRN2", target_bir_lowering=False, debug=False,
                   num_devices=N_CORES)

    xin = nc.dram_tensor("xin", [B, TSH, D], dt.float32r, kind="ExternalInput").ap()
    wt = nc.dram_tensor("wt", [128, DSUB, P], dt.float32r, kind="ExternalInput").ap()
    bb = nc.dram_tensor("bb", [P, 1], dt.float32, kind="ExternalInput").ap()
    idr = nc.dram_tensor("idr", [128, 128], dt.float32r, kind="ExternalInput").ap()
    idf = nc.dram_tensor("idf", [P, P], dt.float32, kind="ExternalInput").ap()

    att_raw = nc.dram_tensor("att_raw", [B, P, TSH], dt.float32, kind="ExternalOutput").ap()
    zc = nc.dram_tensor("zc", [P, B], dt.float32, kind="ExternalOutput").ap()
    pooled_c = nc.dram_tensor("pooled_c", [B, P, D], dt.float32, kind="ExternalOutput").ap()

    with tile.TileContext(nc) as tc:
        with (
            tc.tile_pool(name="consts", bufs=1) as consts,
            tc.tile_pool(name="xin_p", bufs=2) as xin_p,
            tc.tile_pool(name="xt_p", bufs=2) as xt_p,
            tc.tile_pool(name="e_p", bufs=3) as e_p,
            tc.tile_pool(name="et_p", bufs=2) as et_p,
            tc.tile_pool(name="pl_p", bufs=2) as pl_p,
            tc.tile_pool(name="ps_tr", bufs=2, space="PSUM") as ps_tr,
            tc.tile_pool(name="ps_lg", bufs=2, space="PSUM") as ps_lg,
            tc.tile_pool(name="ps_et", bufs=2, space="PSUM") as ps_et,
            tc.tile_pool(name="ps_pl", bufs=1, space="PSUM") as ps_pl,
        ):
            wt_sb = consts.tile([128, DSUB, P], dt.float32r)
            b_sb = consts.tile([P, 1], dt.float32)
            idr_sb = consts.tile([128, 128], dt.float32r)
            idf_sb = consts.tile([P, P], dt.float32)
            z_sb = consts.tile([P, B], dt.float32)
            nc.sync.dma_start(out=wt_sb, in_=wt)
            nc.sync.dma_start(out=b_sb, in_=bb)
            nc.sync.dma_start(out=idr_sb, in_=idr)
            nc.sync.dma_start(out=idf_sb, in_=idf)

            for b in range(B):
                # ---- load x slice naturally: [128 (t), TBLK, D] ----
                x_tile = xin_p.tile([128, TBLK, D], dt.float32r)
                nc.sync.dma_start(
                    out=x_tile,
                    in_=xin[b].rearrange("(tb tp) d -> tp tb d", tp=128),
                )

                # ---- transpose to xT tiles: [128 (d), DSUB, TSH] ----
                xt_tile = xt_p.tile([128, DSUB, TSH], dt.float32r)
                for ds in range(DSUB):
                    pt = ps_tr.tile([128, TSH], dt.float32r)
                    for tb in range(TBLK):
                        nc.tensor.transpose(
                            pt[:, tb * 128:(tb + 1) * 128],
                            x_tile[:, tb, ds * 128:(ds + 1) * 128],
                            idr_sb,
                        )
                    if ds % 2 == 0:
                        nc.scalar.activation(xt_tile[:, ds], pt, AF.Copy)
                    else:
                        nc.vector.tensor_copy(xt_tile[:, ds], pt)

                # ---- mm1: logitsT [P, TSH] ----
                ps_l = ps_lg.tile([P, TSH], dt.float32)
                for ds in range(DSUB):
                    nc.tensor.matmul(ps_l, wt_sb[:, ds], xt_tile[:, ds],
                                     start=(ds == 0), stop=(ds == DSUB - 1))

                # ---- exp(logits + b), Z partial via accum_out ----
                e_sb = e_p.tile([P, TSH], dt.float32)
                nc.scalar.activation(e_sb, ps_l, AF.Exp, bias=b_sb, scale=1.0,
                                     accum_out=z_sb[:, b:b + 1])
                nc.sync.dma_start(out=att_raw[b], in_=e_sb)

                # ---- E chunks back to [t, p] for mm2 stationary ----
                et_sb = et_p.tile([128, TBLK, P], dt.float32r)
                for tb in range(TBLK):
                    pe = ps_et.tile([128, P], dt.float32)
                    nc.tensor.transpose(pe, e_sb[:, tb * 128:(tb + 1) * 128],
                                        idf_sb)
                    nc.vector.tensor_copy(et_sb[:, tb], pe)

                # ---- mm2: pooled_raw [P, D] ----
                ps_p = ps_pl.tile([P, D], dt.float32)
                for tb in range(TBLK):
                    for dh in range(2):
                        nc.tensor.matmul(
                            ps_p[:, dh * 512:(dh + 1) * 512],
                            et_sb[:, tb],
                            x_tile[:, tb, dh * 512:(dh + 1) * 512],
                            start=(tb == 0), stop=(tb == TBLK - 1),
                        )

                pl_sb = pl_p.tile([P, D], dt.float32)
                nc.scalar.activation(pl_sb, ps_p, AF.Copy)
                nc.sync.dma_start(out=pooled_c[b], in_=pl_sb)

            nc.sync.dma_start(out=zc, in_=z_sb)

    nc.compile()
    return nc


def kernel(x, W, b):
    x = np.ascontiguousarray(x, dtype=np.float32)
    W = np.ascontiguousarray(W, dtype=np.float32)
    b = np.ascontiguousarray(b, dtype=np.float32)

    if "nc" not in _CACHE:
        _CACHE["nc"] = _build()
    nc = _CACHE["nc"]

    wt_feed = W.T.reshape(DSUB, 128, P).transpose(1, 0, 2).copy()
    in_maps = []
    for c in range(N_CORES):
        in_maps.append({
            "xin": np.ascontiguousarray(x[:, c * TSH:(c + 1) * TSH, :]),
            "wt": wt_feed,
            "bb": b[:, None].copy(),
            "idr": np.eye(128, dtype=np.float32),
            "idf": np.eye(P, dtype=np.float32),
        })

    _CACHE["in_maps"] = in_maps
    res = run_bass_kernel_spmd(nc, in_maps, core_ids=list(range(N_CORES)))

    att_raw = np.empty((B, P, T), dtype=np.float32)
    Z = np.zeros((B, P), dtype=np.float32)
    pooled = np.zeros((B, P, D), dtype=np.float32)
    for c in range(N_CORES):
        out = res.results[c]
        att_raw[:, :, c * TSH:(c + 1) * TSH] = out["att_raw"]
        Z += np.asarray(out["zc"], dtype=np.float32).T
        pooled += out["pooled_c"]

    att = att_raw / Z[:, :, None]
    pooled = pooled / Z[:, :, None]
    return pooled.astype(np.float32), att.astype(np.float32)


if __name__ == "__main__":
    rng = np.random.default_rng(0)
    x = rng.standard_normal((B, T, D), dtype=np.float32)
    W = (rng.uniform(-1, 1, (P, D)) / 32).astype(np.float32)
    b = (rng.uniform(-1, 1, P) / 32).astype(np.float32)
    pooled, att = kernel(x, W, b)
    print(pooled.shape, att.shape, att.sum(-1)[:2, :2])


# revision 3
# speedup vs baseline: 1.2324x; 1.2324x over previous
"""MultiProbeAttentionPooler Trainium2 kernel.

  logits[b,t,p] = x[b,t,:] @ W[p,:] + b[p]
  att = softmax(logits, axis=t)          -> returned as [B, P, T]
  pooled[b,p,d] = sum_t att[b,t,p] x[b,t,d]

Sharding: T is split across the 8 cores (512 tokens each); every core
handles all 16 batches.  Softmax normalization is deferred to the host:
each core returns unnormalized exp(logits) tiles, per-probe partial sums
Z, and unnormalized partial pooled sums; the host reduces/divides.

The TensorE contracts over the partition dim, so the logits matmul needs
x with D on partitions while the pooling matmul needs T on partitions.
On-chip PE transposes run HAM-throttled at 1.2 GHz and dominate; instead
the host feeds x twice in fp16 (natural + pre-transposed).  Total input
traffic is 32 MB/core - identical to a single fp32 copy - and fp16 moves
through the PE at 1 cycle/row.  fp16's 10-bit mantissa keeps the overall
error at ~5e-4; logits accumulate in fp32 PSUM and the attention output
path (exp, normalization) is fp32 throughout.

Per core, per (batch, T-shard) unit of work:
  - DMA x_nat [512, 1024] and xT [1024, 512] (fp16, 1 MB each).
  - mm1: logitsT[p, t] += WT[dchunk].T @ xT[dchunk]  (8 chunks, fp32 PSUM)
  - ScalarE Exp with per-partition bias: E (fp32, -> att output) plus its
    row-sums Z (accum_out) in one pass.
  - E chunks transposed (tiny [16,128] PE ops) to fp16 [t, p] stationary.
  - mm2: pooled_raw[p, d] += eT[tblk].T @ x_nat[tblk]  (fp32 PSUM).
"""

import sys

import numpy as np

for _p in ("/opt/trn_rl_repo", "/root/.axon_site/_ro/trn_rl_repo"):
    if _p not in sys.path:
        sys.path.append(_p)

import concourse.bacc as bacc
import concourse.tile as tile
from concourse import mybir
from concourse.bass_utils import run_bass_kernel_spmd

dt = mybir.dt
AF = mybir.ActivationFunctionType

B, T, D, P = 16, 4096, 1024, 16
N_CORES = 8
TSH = T // N_CORES          # tokens per core (512)
TBLK = TSH // 128           # 128-token blocks per unit (4)
DSUB = D // 128             # 128-wide D chunks (8)

_CACHE = {}


def _build():
    nc = bacc.Bacc("TRN2", target_bir_lowering=False, debug=False,
                   num_devices=N_CORES)

    xn = nc.dram_tensor("xn", [B, TSH, D], dt.float16, kind="ExternalInput").ap()
    xt = nc.dram_tensor("xt", [B, D, TSH], dt.float16, kind="ExternalInput").ap()
    wt = nc.dram_tensor("wt", [128, DSUB, P], dt.float16, kind="ExternalInput").ap()
    bb = nc.dram_tensor("bb", [P, 1], dt.float32, kind="ExternalInput").ap()
    idf = nc.dram_tensor("idf", [P, P], dt.float32, kind="ExternalInput").ap()

    att_raw = nc.dram_tensor("att_raw", [B, P, TSH], dt.float32, kind="ExternalOutput").ap()
    zc = nc.dram_tensor("zc", [P, B], dt.float32, kind="ExternalOutput").ap()
    pooled_c = nc.dram_tensor("pooled_c", [B, P, D], dt.float32, kind="ExternalOutput").ap()

    with tile.TileContext(nc) as tc:
        with (
            tc.tile_pool(name="consts", bufs=1) as consts,
            tc.tile_pool(name="xn_p", bufs=3) as xn_p,
            tc.tile_pool(name="xt_p", bufs=3) as xt_p,
            tc.tile_pool(name="e_p", bufs=3) as e_p,
            tc.tile_pool(name="et_p", bufs=2) as et_p,
            tc.tile_pool(name="pl_p", bufs=2) as pl_p,
            tc.tile_pool(name="ps_lg", bufs=2, space="PSUM") as ps_lg,
            tc.tile_pool(name="ps_et", bufs=2, space="PSUM") as ps_et,
            tc.tile_pool(name="ps_pl", bufs=2, space="PSUM") as ps_pl,
        ):
            wt_sb = consts.tile([128, DSUB, P], dt.float16)
            b_sb = consts.tile([P, 1], dt.float32)
            idf_sb = consts.tile([P, P], dt.float32)
            z_sb = consts.tile([P, B], dt.float32)
            nc.sync.dma_start(out=wt_sb, in_=wt)
            nc.sync.dma_start(out=b_sb, in_=bb)
            nc.sync.dma_start(out=idf_sb, in_=idf)

            for b in range(B):
                # ---- load x slice both ways (fp16, 1 MB each) ----
                xn_tile = xn_p.tile([128, TBLK, D], dt.float16)
                nc.sync.dma_start(
                    out=xn_tile,
                    in_=xn[b].rearrange("(tb tp) d -> tp tb d", tp=128),
                )
                xt_tile = xt_p.tile([128, DSUB, TSH], dt.float16)
                nc.sync.dma_start(
                    out=xt_tile,
                    in_=xt[b].rearrange("(ds dp) t -> dp ds t", dp=128),
                )

                # ---- mm1: logitsT [P, TSH] ----
                ps_l = ps_lg.tile([P, TSH], dt.float32)
                for ds in range(DSUB):
                    nc.tensor.matmul(ps_l, wt_sb[:, ds], xt_tile[:, ds],
                                     start=(ds == 0), stop=(ds == DSUB - 1))

                # ---- exp(logits + b) -> E (fp32), Z partial via accum_out ----
                e_sb = e_p.tile([P, TSH], dt.float32)
                nc.scalar.activation(e_sb, ps_l, AF.Exp, bias=b_sb, scale=1.0,
                                     accum_out=z_sb[:, b:b + 1])
                nc.sync.dma_start(out=att_raw[b], in_=e_sb)

                # ---- E chunks back to [t, p] (fp16) for mm2 stationary ----
                et_sb = et_p.tile([128, TBLK, P], dt.float16)
                for tb in range(TBLK):
                    pe = ps_et.tile([128, P], dt.float32)
                    nc.tensor.transpose(pe, e_sb[:, tb * 128:(tb + 1) * 128],
                                        idf_sb)
                    nc.vector.tensor_copy(et_sb[:, tb], pe)

                # ---- mm2: pooled_raw [P, D] ----
                ps_p = ps_pl.tile([P, D], dt.float32)
                for tb in range(TBLK):
                    for dh in range(2):
                        nc.tensor.matmul(
                            ps_p[:, dh * 512:(dh + 1) * 512],
                            et_sb[:, tb],
                            xn_tile[:, tb, dh * 512:(dh + 1) * 512],
                            start=(tb == 0), stop=(tb == TBLK - 1),
                        )

                pl_sb = pl_p.tile([P, D], dt.float32)
                nc.scalar.activation(pl_sb, ps_p, AF.Copy)
                nc.sync.dma_start(out=pooled_c[b], in_=pl_sb)

            nc.sync.dma_start(out=zc, in_=z_sb)

    nc.compile()
    return nc


def kernel(x, W, b):
    x = np.ascontiguousarray(x, dtype=np.float32)
    W = np.ascontiguousarray(W, dtype=np.float32)
    b = np.ascontiguousarray(b, dtype=np.float32)

    if "nc" not in _CACHE:
        _CACHE["nc"] = _build()
    nc = _CACHE["nc"]

    x16 = x.astype(np.float16)
    wt_feed = W.T.astype(np.float16).reshape(DSUB, 128, P).transpose(1, 0, 2).copy()
    in_maps = []
    for c in range(N_CORES):
        xs = x16[:, c * TSH:(c + 1) * TSH, :]
        in_maps.append({
            "xn": np.ascontiguousarray(xs),
            "xt": np.ascontiguousarray(xs.transpose(0, 2, 1)),
            "wt": wt_feed,
            "bb": b[:, None].copy(),
            "idf": np.eye(P, dtype=np.float32),
        })

    _CACHE["in_maps"] = in_maps
    res = run_bass_kernel_spmd(nc, in_maps, core_ids=list(range(N_CORES)))

    att_raw = np.empty((B, P, T), dtype=np.float32)
    Z = np.zeros((B, P), dtype=np.float32)
    pooled = np.zeros((B, P, D), dtype=np.float32)
    for c in range(N_CORES):
        out = res.results[c]
        att_raw[:, :, c * TSH:(c + 1) * TSH] = out["att_raw"]
        Z += np.asarray(out["zc"], dtype=np.float32).T
        pooled += out["pooled_c"]

    att = att_raw / Z[:, :, None]
    pooled = pooled / Z[:, :, None]
    return pooled.astype(np.float32), att.astype(np.float32)


if __name__ == "__main__":
    rng = np.random.default_rng(0)
    x = rng.standard_normal((B, T, D), dtype=np.float32)
    W = (rng.uniform(-1, 1, (P, D)) / 32).astype(np.float32)
    b = (rng.uniform(-1, 1, P) / 32).astype(np.float32)
    pooled, att = kernel(x, W, b)
    print(pooled.shape, att.shape, att.sum(-1)[:2, :2])


# revision 8
# speedup vs baseline: 1.8079x; 1.4670x over previous
"""MultiProbeAttentionPooler Trainium2 kernel.

  logits[b,t,p] = x[b,t,:] @ W[p,:] + b[p]
  att = softmax(logits, axis=t)          -> returned as [B, P, T]
  pooled[b,p,d] = sum_t att[b,t,p] x[b,t,d]

Sharding: T is split across the 8 cores (512 tokens each); every core
handles all 16 batches.  Softmax normalization is deferred to the host:
each core returns unnormalized exp(logits) tiles, per-probe partial sums
Z, and unnormalized partial pooled sums; the host reduces/divides.

The TensorE contracts over the partition dim, so the logits matmul needs
x with D on partitions while the pooling matmul needs T on partitions.
On-chip PE transposes run HAM-throttled at 1.2 GHz and dominate; instead
the host feeds x twice in fp16 (natural + pre-transposed).  Total input
traffic is 32 MB/core - identical to a single fp32 copy - and fp16 moves
through the PE at 1 cycle/row.  fp16's 10-bit mantissa keeps the overall
error at ~5e-4; logits accumulate in fp32 PSUM and the attention output
path (exp, normalization) is fp32 throughout.

Per core, per (batch, T-shard) unit of work:
  - DMA x_nat [512, 1024] and xT [1024, 512] (fp16, 1 MB each).
  - mm1: logitsT[p, t] += WT[dchunk].T @ xT[dchunk]  (8 chunks, fp32 PSUM)
  - ScalarE Exp with per-partition bias: E (fp32, -> att output) plus its
    row-sums Z (accum_out) in one pass.
  - E chunks transposed (tiny [16,128] PE ops) to fp16 [t, p] stationary.
  - mm2: pooled_raw[p, d] += eT[tblk].T @ x_nat[tblk]  (fp32 PSUM).
"""

import sys

import numpy as np

for _p in ("/opt/trn_rl_repo", "/root/.axon_site/_ro/trn_rl_repo"):
    if _p not in sys.path:
        sys.path.append(_p)

import concourse.bacc as bacc
import concourse.tile as tile
from concourse import mybir
from concourse.bass_utils import run_bass_kernel_spmd

dt = mybir.dt
AF = mybir.ActivationFunctionType

B, T, D, P = 16, 4096, 1024, 16
N_CORES = 8
TSH = T // N_CORES          # tokens per core (512)
TBLK = TSH // 128           # 128-token blocks per unit (4)
DSUB = D // 128             # 128-wide D chunks (8)

_CACHE = {}


def _build():
    nc = bacc.Bacc("TRN2", target_bir_lowering=False, debug=False,
                   num_devices=N_CORES)

    # pre-tiled on host: each SBUF partition reads one contiguous 8 KB run
    xn = nc.dram_tensor("xn", [B, 128, TBLK, D], dt.float16, kind="ExternalInput").ap()
    xt = nc.dram_tensor("xt", [B, 128, DSUB, TSH], dt.float16, kind="ExternalInput").ap()
    wt = nc.dram_tensor("wt", [128, DSUB, P], dt.float16, kind="ExternalInput").ap()
    bb = nc.dram_tensor("bb", [P, 1], dt.float32, kind="ExternalInput").ap()
    idf = nc.dram_tensor("idf", [P, P], dt.float32, kind="ExternalInput").ap()

    att_raw = nc.dram_tensor("att_raw", [B, P, TSH], dt.float32, kind="ExternalOutput").ap()
    zc = nc.dram_tensor("zc", [P, B], dt.float32, kind="ExternalOutput").ap()
    pooled_c = nc.dram_tensor("pooled_c", [B, P, D], dt.float32, kind="ExternalOutput").ap()

    with tile.TileContext(nc) as tc:
        with (
            tc.tile_pool(name="consts", bufs=1) as consts,
            tc.tile_pool(name="xn_p", bufs=3) as xn_p,
            tc.tile_pool(name="xt_p", bufs=3) as xt_p,
            tc.tile_pool(name="e_p", bufs=3) as e_p,
            tc.tile_pool(name="et_p", bufs=2) as et_p,
            tc.tile_pool(name="pl_p", bufs=2) as pl_p,
            tc.tile_pool(name="ps_lg", bufs=2, space="PSUM") as ps_lg,
            tc.tile_pool(name="ps_et", bufs=2, space="PSUM") as ps_et,
            tc.tile_pool(name="ps_pl", bufs=2, space="PSUM") as ps_pl,
        ):
            wt_sb = consts.tile([128, DSUB, P], dt.float16)
            b_sb = consts.tile([P, 1], dt.float32)
            idf_sb = consts.tile([P, P], dt.float32)
            z_sb = consts.tile([P, B], dt.float32)
            nc.sync.dma_start(out=wt_sb, in_=wt)
            nc.sync.dma_start(out=b_sb, in_=bb)
            nc.sync.dma_start(out=idf_sb, in_=idf)

            for b in range(B):
                # ---- load x slice both ways (fp16, 1 MB each) ----
                # separate HWDGE rings: xn via SP, xt via ACT
                xn_tile = xn_p.tile([128, TBLK, D], dt.float16)
                nc.sync.dma_start(out=xn_tile, in_=xn[b])
                xt_tile = xt_p.tile([128, DSUB, TSH], dt.float16)
                nc.scalar.dma_start(out=xt_tile, in_=xt[b])

                # ---- mm1: logitsT [P, TSH] ----
                ps_l = ps_lg.tile([P, TSH], dt.float32)
                for ds in range(DSUB):
                    nc.tensor.matmul(ps_l, wt_sb[:, ds], xt_tile[:, ds],
                                     start=(ds == 0), stop=(ds == DSUB - 1))

                # ---- exp(logits + b) -> E (fp32), Z partial via accum_out ----
                e_sb = e_p.tile([P, TSH], dt.float32)
                nc.scalar.activation(e_sb, ps_l, AF.Exp, bias=b_sb, scale=1.0,
                                     accum_out=z_sb[:, b:b + 1])
                nc.scalar.dma_start(out=att_raw[b], in_=e_sb)

                # ---- E chunks back to [t, p] (fp16) for mm2 stationary ----
                et_sb = et_p.tile([128, TBLK, P], dt.float16)
                for tb in range(TBLK):
                    pe = ps_et.tile([128, P], dt.float32)
                    nc.tensor.transpose(pe, e_sb[:, tb * 128:(tb + 1) * 128],
                                        idf_sb)
                    nc.vector.tensor_copy(et_sb[:, tb], pe)

                # ---- mm2: pooled_raw [P, D] ----
                ps_p = ps_pl.tile([P, D], dt.float32)
                for tb in range(TBLK):
                    for dh in range(2):
                        nc.tensor.matmul(
                            ps_p[:, dh * 512:(dh + 1) * 512],
                            et_sb[:, tb],
                            xn_tile[:, tb, dh * 512:(dh + 1) * 512],
                            start=(tb == 0), stop=(tb == TBLK - 1),
                        )

                pl_sb = pl_p.tile([P, D], dt.float32)
                nc.vector.tensor_copy(pl_sb, ps_p)
                nc.sync.dma_start(out=pooled_c[b], in_=pl_sb)

            nc.sync.dma_start(out=zc, in_=z_sb)

    nc.compile()
    return nc


def kernel(x, W, b):
    x = np.ascontiguousarray(x, dtype=np.float32)
    W = np.ascontiguousarray(W, dtype=np.float32)
    b = np.ascontiguousarray(b, dtype=np.float32)

    if "nc" not in _CACHE:
        _CACHE["nc"] = _build()
    nc = _CACHE["nc"]

    x16 = x.astype(np.float16)
    wt_feed = W.T.astype(np.float16).reshape(DSUB, 128, P).transpose(1, 0, 2).copy()
    in_maps = []
    for c in range(N_CORES):
        xs = x16[:, c * TSH:(c + 1) * TSH, :]            # [B, TSH, D]
        # xn_feed[b, tp, tb, d] = xs[b, tb*128+tp, d]
        xn_feed = np.ascontiguousarray(
            xs.reshape(B, TBLK, 128, D).transpose(0, 2, 1, 3))
        # xt_feed[b, dp, ds, t] = xs[b, t, ds*128+dp]
        xt_feed = np.ascontiguousarray(
            xs.reshape(B, TSH, DSUB, 128).transpose(0, 3, 2, 1))
        in_maps.append({
            "xn": xn_feed,
            "xt": xt_feed,
            "wt": wt_feed,
            "bb": b[:, None].copy(),
            "idf": np.eye(P, dtype=np.float32),
        })

    _CACHE["in_maps"] = in_maps
    res = run_bass_kernel_spmd(nc, in_maps, core_ids=list(range(N_CORES)))

    att_raw = np.empty((B, P, T), dtype=np.float32)
    Z = np.zeros((B, P), dtype=np.float32)
    pooled = np.zeros((B, P, D), dtype=np.float32)
    for c in range(N_CORES):
        out = res.results[c]
        att_raw[:, :, c * TSH:(c + 1) * TSH] = out["att_raw"]
        Z += np.asarray(out["zc"], dtype=np.float32).T
        pooled += out["pooled_c"]

    att = att_raw / Z[:, :, None]
    pooled = pooled / Z[:, :, None]
    return pooled.astype(np.float32), att.astype(np.float32)


if __name__ == "__main__":
    rng = np.random.default_rng(0)
    x = rng.standard_normal((B, T, D), dtype=np.float32)
    W = (rng.uniform(-1, 1, (P, D)) / 32).astype(np.float32)
    b = (rng.uniform(-1, 1, P) / 32).astype(np.float32)
    pooled, att = kernel(x, W, b)
    print(pooled.shape, att.shape, att.sum(-1)[:2, :2])
